# revision 6
# baseline (speedup 1.0000x reference)
"""Multi-head attention (B=2, S=2048, D=1024, H=16) on 8 Trainium2 cores.

Sharding: tensor-parallel over heads — 2 heads per core. Each core computes
QKV for its heads (contraction over D with the full x), per-head attention
(scores -> exp -> AV, softmax without max-subtraction since scores are O(1)),
and a partial output projection against its 128 rows of Wproj. The host sums
the 8 partial projections (the "all-reduce") and adds bproj.

Device layouts (per core):
  xT   (1024, 4096)  x transposed (D on partitions)
  wqk  (1024, 256)   [Wq_local | Wk_local], pre-scaled by HD**-0.25
  wv   (1024, 128)   Wv_local
  wp   (128, 1024)   Wproj rows for the local heads
  bias (128, 3)      [bq*s | bk*s | bv] per local feature
  cst  (1, 64)       ones
  out  (4096, 1024)  partial projection output

All matmuls run as float32r (TF32-class, full PE rate at free dim >= 256).
QKV biases are folded into the PSUM-drain tensor_scalar add. The softmax
denominator comes free from ones columns bracketing v ([1|vA|vB|1]); the
per-query reciprocal is broadcast across partitions with a K=1 matmul.

Schedule: one unified PSUM pool (tags mm=2, s=2x(128,1024), small=2 banks)
so batch-0 attention overlaps batch-1 QKV with no phase barriers.
"""

import numpy as np

B, S, D, H = 2, 2048, 1024, 16
HD = D // H          # 64
T = B * S            # 4096 tokens
N_CORES = 8
HPC = H // N_CORES   # 2 heads per core

KD = [(i * 128, 128) for i in range(8)]  # contraction chunks over D

_NC_CACHE = {}


def build_nc(reps: int = 1):
    """Build (and bacc-compile) the Bass program. `reps` repeats the whole
    body back-to-back inside one NEFF for timing runs."""
    key = reps
    if key in _NC_CACHE:
        return _NC_CACHE[key]

    from concourse import bacc
    import concourse.bass as bass
    import concourse.mybir as mybir
    import concourse.tile as tile
    from concourse.masks import make_identity

    F32 = mybir.dt.float32
    F32R = mybir.dt.float32r
    Exp = mybir.ActivationFunctionType.Exp
    ADD = mybir.AluOpType.add

    nc = bacc.Bacc()
    xT = nc.declare_dram_parameter("xT", [D, T], F32, isOutput=False)
    wqk = nc.declare_dram_parameter("wqk", [D, 256], F32, isOutput=False)
    wv = nc.declare_dram_parameter("wv", [D, 128], F32, isOutput=False)
    wp = nc.declare_dram_parameter("wp", [128, D], F32, isOutput=False)
    bias = nc.declare_dram_parameter("bias", [128, 3], F32, isOutput=False)
    cst = nc.declare_dram_parameter("cst", [1, 64], F32, isOutput=False)
    out = nc.declare_dram_parameter("out", [T, D], F32, isOutput=True)

    with tile.TileContext(nc) as tc:
        with (
            tc.tile_pool(name="persist", bufs=1) as persist,
            tc.tile_pool(name="wpool", bufs=1) as wpool,
            tc.tile_pool(name="xtp", bufs=3) as xtp,
            tc.tile_pool(name="vtp", bufs=2) as vtp,
            tc.tile_pool(name="expp", bufs=4) as expp,
            tc.tile_pool(name="smp", bufs=2) as smp,
            tc.tile_pool(name="osbp", bufs=3) as osbp,
            tc.tile_pool(name="drp", bufs=4, space="DRAM") as drp,
            tc.tile_pool(name="psum", bufs=2, space="PSUM") as psum,
        ):
            ident = persist.tile([128, 128], F32, tag="ident")
            make_identity(nc, ident)
            ones64 = persist.tile([1, 64], F32R, tag="ones64")
            nc.sync.dma_start(out=ones64, in_=cst[0:1, :].bitcast(F32R))
            bias_sb = persist.tile([128, 3], F32, tag="bias_sb")
            nc.sync.dma_start(out=bias_sb, in_=bias[:, :])

            wp_sb = persist.tile([128, D], F32R, tag="wp_sb")
            nc.sync.dma_start(out=wp_sb, in_=wp[:, :].bitcast(F32R))

            qT = persist.tile([128, T], F32R, tag="qT")
            kT = persist.tile([128, T], F32R, tag="kT")
            # v3[:, g, :] = [vA | 1 | vB | 1] for global 128-token block g:
            # head h uses cols 65h:65h+65 ([v|1]); AV output row 64 = sumexp.
            v3 = persist.tile([128, T // 128, 130], F32R, tag="v3")
            for col in (64, 129):
                csrc = bass.AP(
                    tensor=cst[0:1, 0:32].tensor,
                    offset=cst[0:1, 0:32].offset,
                    ap=[[0, 128], [1, T // 128], [0, 1]],
                )
                nc.sync.dma_start(
                    out=v3[:, :, col : col + 1], in_=csrc.bitcast(F32R)
                )
            aot = [
                persist.tile([128, S], F32R, tag=f"aot{b}", name=f"aot{b}")
                for b in range(B)
            ]

            w_qk, w_v = [], []
            for i, (ofs, ksz) in enumerate(KD):
                wq_t = wpool.tile([ksz, 256], F32R, tag=f"wqk{i}", name=f"wqk{i}")
                nc.sync.dma_start(
                    out=wq_t, in_=wqk[ofs : ofs + ksz, :].bitcast(F32R)
                )
                w_qk.append(wq_t)
                wv_t = wpool.tile([ksz, 128], F32R, tag=f"wv{i}", name=f"wv{i}")
                nc.sync.dma_start(
                    out=wv_t, in_=wv[ofs : ofs + ksz, :].bitcast(F32R)
                )
                w_v.append(wv_t)

            def emit_qkv_tc(tci):
                """QKV + v-transpose for one 512-token strip."""
                c0 = tci * 512
                xts = []
                for i, (ofs, ksz) in enumerate(KD):
                    xt_t = xtp.tile([ksz, 512], F32R, tag=f"x{i}", name=f"x{i}")
                    nc.sync.dma_start(
                        out=xt_t,
                        in_=xT[ofs : ofs + ksz, c0 : c0 + 512].bitcast(F32R),
                    )
                    xts.append(xt_t)
                for m, dst in ((0, qT), (1, kT)):
                    pqk = psum.tile([128, 512], F32, tag="mm", name="pqk")
                    for i in range(len(KD)):
                        nc.tensor.matmul(
                            pqk,
                            lhsT=w_qk[i][:, m * 128 : (m + 1) * 128],
                            rhs=xts[i],
                            start=(i == 0),
                            stop=(i == len(KD) - 1),
                        )
                    with nc.allow_low_precision(reason="f32r qkv tiles"):
                        nc.vector.tensor_scalar(
                            out=dst[:, c0 : c0 + 512],
                            in0=pqk,
                            scalar1=bias_sb[:, m : m + 1],
                            scalar2=None,
                            op0=ADD,
                        )
                pv = psum.tile([128, 512], F32, tag="mm", name="pv")
                for i in range(len(KD)):
                    nc.tensor.matmul(
                        pv,
                        lhsT=w_v[i],
                        rhs=xts[i],
                        start=(i == 0),
                        stop=(i == len(KD) - 1),
                    )
                vT_t = vtp.tile([128, 512], F32, tag="vt", name="vT_t")
                with nc.allow_low_precision(reason="v bias fold"):
                    nc.vector.tensor_scalar(
                        out=vT_t,
                        in0=pv,
                        scalar1=bias_sb[:, 2:3],
                        scalar2=None,
                        op0=ADD,
                    )
                for s4 in range(4):
                    ptr = psum.tile([128, 128], F32, tag="mm", name="ptr")
                    nc.tensor.transpose(
                        ptr, vT_t[:, s4 * 128 : (s4 + 1) * 128], ident
                    )
                    g = tci * 4 + s4
                    nc.scalar.copy(out=v3[:, g, 0:64], in_=ptr[:, 0:64])
                    nc.scalar.copy(out=v3[:, g, 65:129], in_=ptr[:, 64:128])

            def emit_attn_pair(b, qc):
                """scores->exp->AV->normalize for one (batch, 512-query
                strip), both local heads packed: head A runs on PE rows
                0-63, head B on rows 64-127 (concurrent row-groups)."""
                q0 = b * S + qc * 512
                avs = []
                for h in range(HPC):
                    av = psum.tile([65, 512], F32, tag="small", name=f"av{h}")
                    avs.append(av)
                for kb in range(16):
                    k0 = b * S + kb * 128
                    pscore = psum.tile([128, 1024], F32, tag="s", name="pscore")
                    for h in range(HPC):
                        hr = slice(h * 64, (h + 1) * 64)
                        nc.tensor.matmul(
                            pscore[:, h * 512 : (h + 1) * 512],
                            lhsT=kT[hr, k0 : k0 + 128],
                            rhs=qT[hr, q0 : q0 + 512],
                            start=True,
                            stop=True,
                        )
                    ex = expp.tile([128, 1024], F32R, tag="exp", name="ex")
                    nc.scalar.activation(out=ex, in_=pscore, func=Exp)
                    g = b * 16 + kb
                    for h in range(HPC):
                        nc.tensor.matmul(
                            avs[h],
                            lhsT=v3[:, g, 65 * h : 65 * h + 65],
                            rhs=ex[:, h * 512 : (h + 1) * 512],
                            start=(kb == 0),
                            stop=(kb == 15),
                        )
                for h in range(HPC):
                    hr = slice(h * 64, (h + 1) * 64)
                    rc = smp.tile([1, 512], F32R, tag="rc", name="rc")
                    with nc.allow_low_precision(reason="softmax reciprocal"):
                        nc.vector.reciprocal(out=rc, in_=avs[h][64:65, :])
                    # broadcast rc across 64 partitions via a DRAM bounce
                    scr = drp.tile([1, 512], F32R, tag="scr", name="scr")
                    nc.sync.dma_start(out=scr, in_=rc)
                    bcs = smp.tile([64, 512], F32, tag="bcs", name="bcs")
                    bsrc = bass.AP(
                        tensor=scr.tensor,
                        offset=scr.offset,
                        ap=[[0, 64], [1, 512]],
                    )
                    nc.sync.dma_start(out=bcs, in_=bsrc.bitcast(F32))
                    with nc.allow_low_precision(reason="f32r attn out"):
                        nc.vector.tensor_mul(
                            out=aot[b][hr, qc * 512 : (qc + 1) * 512],
                            in0=avs[h][0:64, :],
                            in1=bcs,
                        )

            def emit_proj(b, qc):
                """partial output projection for one 512-token strip."""
                for t4 in range(4):
                    col0 = qc * 512 + t4 * 128
                    osb = osbp.tile([128, D], F32, tag="osb", name="osb")
                    for n2 in range(2):
                        pp = psum.tile([128, 512], F32, tag="mm", name="pp")
                        nc.tensor.matmul(
                            pp,
                            lhsT=aot[b][:, col0 : col0 + 128],
                            rhs=wp_sb[:, n2 * 512 : (n2 + 1) * 512],
                            start=True,
                            stop=True,
                        )
                        nc.vector.tensor_copy(
                            out=osb[:, n2 * 512 : (n2 + 1) * 512], in_=pp
                        )
                    nc.sync.dma_start(
                        out=out[b * S + col0 : b * S + col0 + 128, :], in_=osb
                    )

            for _ in range(reps):
                # batch-0 QKV, then batch-0 attention interleaved with
                # batch-1 QKV, then batch-1 attention.
                for tci in range(4):
                    emit_qkv_tc(tci)
                for qc in range(4):
                    emit_attn_pair(0, qc)
                    emit_qkv_tc(4 + qc)
                    emit_proj(0, qc)
                for qc in range(4):
                    emit_attn_pair(1, qc)
                    emit_proj(1, qc)

    nc.compile()
    _NC_CACHE[key] = nc
    return nc


def prep_inputs(x, Wqkv, bqkv, Wproj):
    """Host-side sharding: returns the per-core input maps."""
    x = np.asarray(x, dtype=np.float32)
    Wqkv = np.asarray(Wqkv, dtype=np.float32)
    bqkv = np.asarray(bqkv, dtype=np.float32)
    Wproj = np.asarray(Wproj, dtype=np.float32)

    xT = np.ascontiguousarray(x.reshape(T, D).T)
    qk_scale = np.float32(HD ** -0.25)
    cst = np.ones((1, 64), dtype=np.float32)

    in_maps = []
    for c in range(N_CORES):
        f0 = c * HPC * HD  # first local feature column
        wqk_c = np.concatenate(
            [Wqkv[:, f0 : f0 + 128], Wqkv[:, D + f0 : D + f0 + 128]], axis=1
        ) * qk_scale
        wv_c = Wqkv[:, 2 * D + f0 : 2 * D + f0 + 128]
        wp_c = Wproj[f0 : f0 + 128, :]
        bias_c = np.stack(
            [
                bqkv[f0 : f0 + 128] * qk_scale,
                bqkv[D + f0 : D + f0 + 128] * qk_scale,
                bqkv[2 * D + f0 : 2 * D + f0 + 128],
            ],
            axis=1,
        )
        in_maps.append(
            {
                "xT": xT,
                "wqk": np.ascontiguousarray(wqk_c),
                "wv": np.ascontiguousarray(wv_c),
                "wp": np.ascontiguousarray(wp_c),
                "bias": np.ascontiguousarray(bias_c),
                "cst": cst,
            }
        )
    return in_maps


def kernel(x, Wqkv, bqkv, Wproj, bproj):
    from concourse.bass_utils import run_bass_kernel_spmd

    nc = build_nc()
    in_maps = prep_inputs(x, Wqkv, bqkv, Wproj)
    res = run_bass_kernel_spmd(nc, in_maps, core_ids=list(range(N_CORES)))
    total = res.results[0]["out"].astype(np.float32).copy()
    for c in range(1, N_CORES):
        total += res.results[c]["out"]
    total += np.asarray(bproj, dtype=np.float32)[None, :]
    return total.reshape(B, S, D)


if __name__ == "__main__":
    rng = np.random.default_rng(0)
    x = rng.standard_normal((B, S, D)).astype(np.float32)
    Wqkv = (rng.standard_normal((D, 3 * D)) * D**-0.5).astype(np.float32)
    bqkv = np.zeros(3 * D, np.float32)
    Wproj = (rng.standard_normal((D, D)) * D**-0.5).astype(np.float32)
    bproj = np.zeros(D, np.float32)
    got = kernel(x, Wqkv, bqkv, Wproj, bproj)
    print("ran ok", got.shape, got.dtype)
